# revision 1
# baseline (speedup 1.0000x reference)
"""Trainium2 Bass kernel for nn_Decoder_22196390985918 (SPADE-style decoder).

Sharding: 8 cores = (batch b in 0..3) x (H-half in 0..1). Each core computes
out[b, :, h0:h0+64, :] for h0 = 64*(core%2).

Key algorithmic transform: the [B, 512, H, W] "middle" tensor (masked scatter
of per-region style vectors mu[b,j,:]) is never materialized. Since
middle[b,:,h,w] = mu[b, j*(h,w), :] with j* the last active region,
conv(middle) collapses to a conv over the 5 one-hot region masks sel_j with
per-batch tap tables G[j, cc, tap] = sum_k Wconv[cc, k, tap] * mu[b, j, k].
That turns ~77 GFLOP of 512-channel convs into one K=45 matmul per tile.

The SPADE branch (mask -> shared 3x3 conv -> relu -> gamma/beta 3x3 convs) is
computed directly: shared conv via K=27 im2col, gamma/beta convs as 9
accumulating K=128 taps with gamma and beta fused into one M=128 output.
The sigmoid blending factors are folded into the conv weights and biases.

All conv/table matmuls run in float32r (TF32-like); everything else is fp32.
Each im2col is built by a single multi-dim-AP DMA per output chunk; DMA
issue is spread across the sync/tensor/scalar/gpsimd queues.
"""
import os as _os

import numpy as np

import concourse.bacc as bacc
import concourse.bass as bass
import concourse.mybir as mybir
import concourse.tile as tile
from concourse.bass_utils import run_bass_kernel_spmd

dt = mybir.dt
F32 = dt.float32
F32R = dt.float32 if _os.environ.get("KF32") == "1" else dt.float32r
AF = mybir.ActivationFunctionType
ALU = mybir.AluOpType

B, C, H, W, F, L, NH = 4, 64, 128, 128, 5, 512, 128
GW = 130                    # padded grid width  (image col = grid col - 1)
SR = 66                     # seg/sel/actv grid rows (image row = h0 - 1 + r)
MR = 68                     # mask grid rows (image row = h0 - 2 + r)
SEG_N = SR * GW             # 8580
MASK_N = MR * GW            # 8840
SEG_SZ = SEG_N + 2 * GW + 2 + 520   # sel tail slack for im2col windows
MASK_SZ = MASK_N + 2 * GW + 2 + 390
ROWS = 64                   # output rows per core
NCH = 16                    # main conv chunks (4 rows x 128 cols, N=512)
ACH = 22                    # shared conv chunks (3 rows x 128 cols, N=384)
NCORES = 8


def _win_ap(base_ap, flat):
    """9-tap im2col source view: partitions from base_ap, free dims
    (ty[3] x tx[3] x flat window) as overlapping strided windows."""
    return bass.AP(tensor=base_ap.tensor, offset=base_ap.offset,
                   ap=[base_ap.ap[0], [GW, 3], [1, 3], [1, flat]])


def _build_nc():
    lvl = int(_os.environ.get("KSEC", "8"))
    nc = bacc.Bacc()

    # ---- per-core DRAM inputs -------------------------------------------
    xb = nc.dram_tensor("xb", [C, H * W], F32, kind="ExternalInput")
    xown = nc.dram_tensor("xown", [C, ROWS * W], F32, kind="ExternalInput")
    segg = nc.dram_tensor("segg", [F, SEG_N + 264], F32, kind="ExternalInput")
    maskg = nc.dram_tensor("maskg", [3, MASK_N + 264], F32, kind="ExternalInput")
    codes = nc.dram_tensor("codes", [F, L], F32, kind="ExternalInput")
    fcw = nc.dram_tensor("fcw", [F, L, L], F32, kind="ExternalInput")
    fcbt = nc.dram_tensor("fcbt", [L, F], F32, kind="ExternalInput")
    cgw = nc.dram_tensor("cgw", [C, L * 9], F32, kind="ExternalInput")
    cbw = nc.dram_tensor("cbw", [C, L * 9], F32, kind="ExternalInput")
    sgw = nc.dram_tensor("sgw", [C, NH * 9], F32, kind="ExternalInput")
    sbw = nc.dram_tensor("sbw", [C, NH * 9], F32, kind="ExternalInput")
    ssw = nc.dram_tensor("ssw", [NH, 27], F32, kind="ExternalInput")
    cgb = nc.dram_tensor("cgb", [C, 1], F32, kind="ExternalInput")
    cbb = nc.dram_tensor("cbb", [C, 1], F32, kind="ExternalInput")
    sgbb = nc.dram_tensor("sgbb", [C, 1], F32, kind="ExternalInput")
    sbbb = nc.dram_tensor("sbbb", [C, 1], F32, kind="ExternalInput")
    ssb = nc.dram_tensor("ssb", [NH, 1], F32, kind="ExternalInput")
    bg = nc.dram_tensor("bg", [1, 1], F32, kind="ExternalInput")
    bb = nc.dram_tensor("bb", [1, 1], F32, kind="ExternalInput")
    u5 = nc.dram_tensor("u5", [45, 45], F32, kind="ExternalInput")
    ident = nc.dram_tensor("ident", [128, 128], F32, kind="ExternalInput")
    zz = nc.dram_tensor("zz", [128, 652], F32, kind="ExternalInput")
    hal = nc.dram_tensor("hal", [128, 2], F32, kind="ExternalInput")
    out_d = nc.dram_tensor("out", [C, NCH, 512], F32, kind="ExternalOutput")

    with tile.TileContext(nc) as tc:
        with (
            tc.tile_pool(name="const", bufs=1) as cst,
            tc.tile_pool(name="wcb", bufs=4) as wcbp,
            tc.tile_pool(name="wct", bufs=4) as wctp,
            tc.tile_pool(name="fcwp", bufs=2) as fcwp,
            tc.tile_pool(name="cbcp", bufs=1) as cbcp,
            tc.tile_pool(name="ttp", bufs=2) as ttp,
            tc.tile_pool(name="xs", bufs=2) as xsp,
            tc.tile_pool(name="gb", bufs=2) as gbp,
            tc.tile_pool(name="xn", bufs=2) as xnp,
            tc.tile_pool(name="ot", bufs=2) as otp,
            tc.tile_pool(name="pmain", bufs=2, space="PSUM") as pmain,
            tc.tile_pool(name="paux", bufs=2, space="PSUM") as paux,
            tc.tile_pool(name="gpsp", bufs=3, space="PSUM") as gpsp,
        ):
            # ---- tiny constants (sync queue head) -----------------------
            u5r = cst.tile([45, 45], F32R)
            nc.sync.dma_start(out=u5r[:], in_=u5[:].bitcast(F32R))
            id_t = cst.tile([128, 128], F32)
            nc.sync.dma_start(out=id_t[:], in_=ident[:])
            id_r = cst.tile([128, 128], F32R)
            nc.sync.dma_start(out=id_r[:], in_=ident[:].bitcast(F32R))
            sswf = cst.tile([NH, 27], F32)
            nc.sync.dma_start(out=sswf[:], in_=ssw[:])
            graw = cst.tile([128, 1], F32)
            nc.sync.dma_start(out=graw[:], in_=bg[:].to_broadcast((128, 1)))
            braw = cst.tile([128, 1], F32)
            nc.sync.dma_start(out=braw[:], in_=bb[:].to_broadcast((128, 1)))
            convb = cst.tile([128, 1], F32)
            nc.sync.dma_start(out=convb[0:64, :], in_=cgb[:])
            nc.sync.dma_start(out=convb[64:128, :], in_=cbb[:])
            spadeb = cst.tile([128, 1], F32)
            nc.sync.dma_start(out=spadeb[0:64, :], in_=sgbb[:])
            nc.sync.dma_start(out=spadeb[64:128, :], in_=sbbb[:])
            ssb_t = cst.tile([NH, 1], F32)
            nc.sync.dma_start(out=ssb_t[:], in_=ssb[:])
            hal_t = cst.tile([128, 2], F32)
            nc.sync.dma_start(out=hal_t[:], in_=hal[:])
            fcbt_sb = cst.tile([128, 4, F], F32)
            for kb in range(4):
                nc.sync.dma_start(out=fcbt_sb[:, kb, :],
                                  in_=fcbt[kb * 128:(kb + 1) * 128, :])

            ones_t = cst.tile([128, 1], F32)
            nc.gpsimd.memset(ones_t[:], 1.0)
            eps_t = cst.tile([C, 1], F32)
            nc.gpsimd.memset(eps_t[:], 1e-5)
            half1 = cst.tile([128, 1], F32)
            nc.gpsimd.memset(half1[0:64, :], 1.0)
            nc.gpsimd.memset(half1[64:128, :], 0.0)
            zsb = cst.tile([128, 132], F32)
            nc.gpsimd.memset(zsb[:], 0.0)

            # blending factors
            gsig = cst.tile([128, 1], F32)
            nc.scalar.activation(gsig[:], graw[:], AF.Sigmoid)
            bsig = cst.tile([128, 1], F32)
            nc.scalar.activation(bsig[:], braw[:], AF.Sigmoid)
            gba = cst.tile([128, 1], F32)
            nc.vector.tensor_copy(gba[0:64, :], gsig[0:64, :])
            nc.vector.tensor_copy(gba[64:128, :], bsig[64:128, :])
            om_gba = cst.tile([128, 1], F32)
            nc.scalar.activation(om_gba[:], gba[:], AF.Identity, bias=ones_t[:], scale=-1.0)
            tb1 = cst.tile([128, 1], F32)
            nc.vector.tensor_mul(tb1[:], convb[:], gba[:])
            tb2 = cst.tile([128, 1], F32)
            nc.vector.tensor_mul(tb2[:], spadeb[:], om_gba[:])
            bias_t = cst.tile([128, 1], F32)
            nc.vector.tensor_add(bias_t[:], tb1[:], tb2[:])
            bias1_t = cst.tile([128, 1], F32)
            nc.vector.tensor_add(bias1_t[:], bias_t[:], half1[:])

            # ---- big weight loads: wcb (PE-critical) then fw (scalar q) -
            wcbs = []
            for kb in range(4):
                wcb = wcbp.tile([128, 1152], F32, tag="wcb", name=f"wcb{kb}")
                nc.sync.dma_start(out=wcb[0:64, :], in_=cgw[:, kb * 1152:(kb + 1) * 1152])
                nc.sync.dma_start(out=wcb[64:128, :], in_=cbw[:, kb * 1152:(kb + 1) * 1152])
                wcbs.append(wcb)
            sgb = cst.tile([128, 1152], F32)
            nc.sync.dma_start(out=sgb[0:64, :], in_=sgw[:])
            nc.sync.dma_start(out=sgb[64:128, :], in_=sbw[:])

            # ---- grids: pre-shifted replicated loads (gpsimd queue) -----
            sel45 = cst.tile([45, SEG_N], F32R)
            segp = segg[:].ap[0][0]
            for ty in range(3):
                src = bass.AP(tensor=segg[:].tensor, offset=ty * GW,
                              ap=[[1, 3], [segp, F], [1, SEG_N]])
                nc.gpsimd.dma_start(out=sel45[15 * ty:15 * ty + 15, :],
                                    in_=src.bitcast(F32R))
            mask27 = cst.tile([27, MASK_N], F32R)
            maskp_ = maskg[:].ap[0][0]
            for ty in range(3):
                src = bass.AP(tensor=maskg[:].tensor, offset=ty * GW,
                              ap=[[1, 3], [maskp_, 3], [1, MASK_N]])
                nc.gpsimd.dma_start(out=mask27[9 * ty:9 * ty + 9, :],
                                    in_=src.bitcast(F32R))

            # ---- region masks part 1: cnt (PE) -> t (ACT, parked in SBUF)
            t_sb = cst.tile([45, SEG_N], dt.bfloat16)
            segchunks = []
            off = 0 if lvl >= 2 else SEG_N
            while off < SEG_N:
                n = min(512, SEG_N - off)
                segchunks.append((off, n))
                off += n
            for off, n in segchunks:
                pc = paux.tile([45, 512], F32, tag="aux")
                nc.tensor.matmul(pc[:, 0:n], u5r[:], sel45[:, off:off + n],
                                 start=True, stop=True)
                nc.scalar.activation(t_sb[:, off:off + n], pc[:, 0:n], AF.Relu,
                                     bias=ones_t[0:45, :], scale=-1.0)

            # ---- shared conv (mask 3 -> NH), pre-shifted rows -----------
            if lvl >= 3:
                ptp = paux.tile([27, 128], F32, tag="aux")
                nc.tensor.transpose(ptp[:], sswf[:], id_t[:])
                sswT = cst.tile([27, 128], F32R)
                nc.scalar.activation(sswT[:], ptp[:], AF.Copy)

                actv = cst.tile([NH, SR, GW], F32R)
                bord = actv[:, :, 0:1]
                nc.vector.tensor_copy(
                    bass.AP(tensor=bord.tensor, offset=bord.offset,
                            ap=[bord.ap[0], [GW, SR], [GW - 1, 2]]),
                    zsb[:].rearrange("p (a b) -> p a b", a=SR))
                m3 = mask27[:].rearrange("p (r c) -> p r c", c=GW)
                for a in range(ACH):
                    r = 3 * a
                    psh = paux.tile([NH, 3, 128], F32, tag="aux")
                    nc.tensor.matmul(psh[:], sswT[:], m3[:, r:r + 3, 0:128],
                                     start=True, stop=True)
                    nc.scalar.activation(actv[:, r:r + 3, 1:129], psh[:], AF.Relu,
                                         bias=ssb_t[:], scale=1.0)

            # ---- spade gamma/beta lhsT ----------------------------------
            if lvl >= 6:
                nc.vector.tensor_scalar_mul(sgb[:], sgb[:], om_gba[:])
                spT = cst.tile([128, 9, 128], F32R)
                sgb3 = sgb[:].rearrange("p (l t) -> p l t", t=9)
                for t in range(9):
                    pt = paux.tile([128, 128], F32, tag="aux")
                    nc.tensor.transpose(pt[:], sgb3[:, :, t], id_t[:])
                    nc.scalar.activation(spT[:, t, :], pt[:], AF.Copy)

            # ---- mu path (fw on scalar queue; muls split DVE/Pool) ------
            if lvl >= 4:
                z_sb = cst.tile([128, 4, F], F32)
                muT = cst.tile([128, 4, F], F32R)
                for j in range(F):
                    cbc = cbcp.tile([128, L], F32, tag="cbc")
                    nc.scalar.dma_start(out=cbc[:],
                                        in_=codes[j:j + 1, :].to_broadcast((128, L)))
                    eng = nc.vector if j < 3 else nc.gpsimd
                    for kb in range(4):
                        fw = fcwp.tile([128, L], F32, tag="fcw")
                        nc.scalar.dma_start(out=fw[:], in_=fcw[j, kb * 128:(kb + 1) * 128, :])
                        tts = ttp.tile([128, L], F32, tag="tts")
                        eng.tensor_mul(tts[:], fw[:], cbc[:])
                        nc.vector.reduce_sum(out=z_sb[:, kb, j:j + 1], in_=tts[:],
                                             axis=mybir.AxisListType.X)
                for kb in range(4):
                    nc.vector.tensor_add(z_sb[:, kb, :], z_sb[:, kb, :],
                                         fcbt_sb[:, kb, :])
                for kb in range(4):
                    nc.scalar.activation(muT[:, kb, :], z_sb[:, kb, :], AF.Relu)

            # ---- wct transposes (PE) + copies (ACT) ---------------------
            if lvl >= 5:
                wcts = []
                for kb in range(4):
                    wcb = wcbs[kb]
                    nc.vector.tensor_scalar_mul(wcb[:], wcb[:], gba[:])
                    wct = wctp.tile([128, 9, 128], F32R, tag="wct", name=f"wct{kb}")
                    wcb3 = wcb[:].rearrange("p (l t) -> p l t", t=9)
                    for t in range(9):
                        pt = paux.tile([128, 128], F32, tag="aux")
                        nc.tensor.transpose(pt[:], wcb3[:, :, t], id_t[:])
                        nc.scalar.activation(wct[:, t, :], pt[:], AF.Copy)
                    wcts.append(wct)

            # ---- region masks part 2: sel = seg * t (DVE, after mu) -----
            for off, n in segchunks:
                nc.vector.tensor_mul(sel45[:, off:off + n],
                                     sel45[:, off:off + n].bitcast(F32),
                                     t_sb[:, off:off + n])
            if lvl >= 3:
                nc.vector.tensor_scalar_mul(actv[:, 0, :], actv[:, 0, :].bitcast(F32),
                                            hal_t[:, 0:1])
                nc.vector.tensor_scalar_mul(actv[:, SR - 1, :], actv[:, SR - 1, :].bitcast(F32),
                                            hal_t[:, 1:2])

            # ---- G matmuls -> selG --------------------------------------
            if lvl >= 5:
                gps = [gpsp.tile([F, 3, 128], F32, tag="gps", name=f"gps{_g}")
                       for _g in range(3)]
                for kb in range(4):
                    for g in range(3):
                        nc.tensor.matmul(gps[g][:], muT[:, kb, :],
                                         wcts[kb][:, 3 * g:3 * g + 3, :],
                                         start=(kb == 0), stop=(kb == 3))
                selG = cst.tile([45, 128], F32R)
                gstage = cst.tile([F, 9, 128], F32)
                for g in range(3):
                    nc.scalar.activation(gstage[:, 3 * g:3 * g + 3, :], gps[g][:], AF.Copy)
                for t in range(9):
                    nc.sync.dma_start(out=selG[F * t:F * t + F, :],
                                      in_=gstage[:, t, :].bitcast(F32R))

            # ---- instance-norm stats (xb on sync queue) -----------------
            if lvl >= 7:
                stats_t = cst.tile([C, 32, 6], F32)
                for q in range(16):
                    xt = xsp.tile([C, 2, 512], F32, tag="xs")
                    nc.sync.dma_start(out=xt[:], in_=xb[:, q * 1024:(q + 1) * 1024]
                                      .rearrange("c (k n) -> c k n", k=2))
                    for k in range(2):
                        nc.vector.bn_stats(out=stats_t[:, 2 * q + k, :], in_=xt[:, k, :])
                mv = cst.tile([C, 2], F32)
                nc.vector.bn_aggr(out=mv[:], in_=stats_t[:])
                sd = cst.tile([C, 1], F32)
                nc.scalar.activation(sd[:], mv[:, 1:2], AF.Sqrt, bias=eps_t[:], scale=1.0)
                rstd = cst.tile([C, 1], F32)
                nc.vector.reciprocal(rstd[:], sd[:])
                nbias = cst.tile([C, 1], F32)
                nc.vector.tensor_mul(nbias[:], mv[:, 0:1], rstd[:])
                nc.vector.tensor_scalar_mul(nbias[:], nbias[:], -1.0)

            # ---- main conv + epilogue (epilogue one chunk behind) -------
            if lvl >= 8:
                s3 = sel45[:].rearrange("p (r c) -> p r c", c=GW)
                xt2s, xnts, pms = {}, {}, {}

                def conv_chunk(i):
                    xt2s[i] = xnp.tile([C, 4, 128], F32, tag="xn", name=f"xt2_{i}")
                    nc.gpsimd.dma_start(out=xt2s[i][:],
                                        in_=xown[:, i * 512:(i + 1) * 512].rearrange(
                                            "c (r w) -> c r w", r=4))
                    xnts[i] = otp.tile([C, 4, 128], F32, tag="ot", name=f"xnt_{i}")
                    pm = pmain.tile([128, 4, 128], F32, tag="pm", name=f"pm_{i}")
                    pms[i] = pm
                    for t in range(9):
                        ty, tx = divmod(t, 3)
                        nc.tensor.matmul(pm[:], spT[:, t, :],
                                         actv[:, 4 * i + ty:4 * i + ty + 4, tx:tx + 128],
                                         start=(t == 0), stop=False)
                    nc.tensor.matmul(pm[:], selG[:], s3[:, 4 * i:4 * i + 4, 0:128],
                                     start=False, stop=True)

                def epi_chunk(i):
                    pm = pms.pop(i)
                    gb = gbp.tile([128, 4, 128], F32R, tag="gb", name=f"gb_{i}")
                    nc.scalar.activation(gb[:], pm[:], AF.Identity,
                                         bias=bias1_t[:], scale=1.0)
                    pb = gpsp.tile([64, 4, 128], F32, tag="gps", name=f"pb_{i}")
                    nc.tensor.matmul(pb[:].rearrange("p t c -> p (t c)"), id_r[:, 64:128],
                                     gb[:].rearrange("p t c -> p (t c)"),
                                     start=True, stop=True)
                    xt2, xnt = xt2s.pop(i), xnts[i]
                    nc.gpsimd.tensor_scalar(xnt[:], xt2[:],
                                            rstd[:], nbias[:],
                                            op0=ALU.mult, op1=ALU.add)
                    nc.gpsimd.tensor_mul(xnt[:], xnt[:], gb[0:64, :, :].bitcast(F32))
                    nc.vector.tensor_add(xnt[:].rearrange("p t c -> p (t c)"),
                                         xnt[:].rearrange("p t c -> p (t c)"),
                                         pb[:].rearrange("p t c -> p (t c)"))
                    nc.sync.dma_start(out=out_d[:, i, :],
                                      in_=xnts.pop(i)[:].rearrange("c r w -> c (r w)"))

                conv_chunk(0)
                for i in range(1, NCH):
                    conv_chunk(i)
                    epi_chunk(i - 1)
                epi_chunk(NCH - 1)

    nc.finalize()
    return nc


_NC = None


def kernel(**inputs):
    global _NC
    x = np.asarray(inputs["x"], dtype=np.float32)
    segmap = np.asarray(inputs["segmap"], dtype=np.float32)
    codes_vector = np.asarray(inputs["codes_vector"], dtype=np.float32)
    mask = np.asarray(inputs["mask"], dtype=np.float32)
    fc_w = np.ascontiguousarray(np.asarray(inputs["fc_w"], dtype=np.float32))
    fc_b = np.asarray(inputs["fc_b"], dtype=np.float32)
    conv_gamma_w = np.asarray(inputs["conv_gamma_w"], dtype=np.float32)
    conv_gamma_b = np.asarray(inputs["conv_gamma_b"], dtype=np.float32)
    conv_beta_w = np.asarray(inputs["conv_beta_w"], dtype=np.float32)
    conv_beta_b = np.asarray(inputs["conv_beta_b"], dtype=np.float32)
    spade_shared_w = np.asarray(inputs["spade_shared_w"], dtype=np.float32)
    spade_shared_b = np.asarray(inputs["spade_shared_b"], dtype=np.float32)
    spade_gamma_w = np.asarray(inputs["spade_gamma_w"], dtype=np.float32)
    spade_gamma_b = np.asarray(inputs["spade_gamma_b"], dtype=np.float32)
    spade_beta_w = np.asarray(inputs["spade_beta_w"], dtype=np.float32)
    spade_beta_b = np.asarray(inputs["spade_beta_b"], dtype=np.float32)
    blending_gamma = np.asarray(inputs["blending_gamma"], dtype=np.float32)
    blending_beta = np.asarray(inputs["blending_beta"], dtype=np.float32)

    if _NC is None:
        _NC = _build_nc()

    shared = {
        "fcw": np.ascontiguousarray(fc_w),
        "fcbt": np.ascontiguousarray(fc_b.T),
        "cgw": np.ascontiguousarray(conv_gamma_w.reshape(C, L * 9)),
        "cbw": np.ascontiguousarray(conv_beta_w.reshape(C, L * 9)),
        "sgw": np.ascontiguousarray(spade_gamma_w.reshape(C, NH * 9)),
        "sbw": np.ascontiguousarray(spade_beta_w.reshape(C, NH * 9)),
        "ssw": np.ascontiguousarray(spade_shared_w.transpose(0, 2, 3, 1).reshape(NH, 27)),
        "cgb": conv_gamma_b.reshape(C, 1), "cbb": conv_beta_b.reshape(C, 1),
        "sgbb": spade_gamma_b.reshape(C, 1), "sbbb": spade_beta_b.reshape(C, 1),
        "ssb": spade_shared_b.reshape(NH, 1),
        "bg": blending_gamma.reshape(1, 1), "bb": blending_beta.reshape(1, 1),
        "u5": np.kron(np.eye(9, dtype=np.float32), np.tril(np.ones((F, F), np.float32), -1)),
        "ident": np.eye(128, dtype=np.float32),
        "zz": np.zeros((128, 652), np.float32),
    }

    in_maps = []
    for c in range(NCORES):
        b, half = divmod(c, 2)
        h0 = half * ROWS
        segp = np.zeros((F, SR * GW + 264), np.float32).reshape(F, -1)
        segp2 = np.zeros((F, SR, GW), np.float32)
        r_lo, r_hi = h0 - 1, h0 + ROWS + 1  # exclusive
        s_lo, s_hi = max(r_lo, 0), min(r_hi, H)
        segp2[:, s_lo - r_lo:s_hi - r_lo, 1:129] = segmap[b, :, s_lo:s_hi, :]
        segp[:, 0:SR * GW] = segp2.reshape(F, -1)
        maskp = np.zeros((3, MR * GW + 264), np.float32)
        maskp2 = np.zeros((3, MR, GW), np.float32)
        m_lo, m_hi = h0 - 2, h0 + ROWS + 2
        ms_lo, ms_hi = max(m_lo, 0), min(m_hi, H)
        maskp2[:, ms_lo - m_lo:ms_hi - m_lo, 1:129] = mask[b, :, ms_lo:ms_hi, :]
        maskp[:, 0:MR * GW] = maskp2.reshape(3, -1)
        in_maps.append(dict(
            shared,
            xb=np.ascontiguousarray(x[b].reshape(C, H * W)),
            xown=np.ascontiguousarray(x[b, :, h0:h0 + ROWS, :].reshape(C, ROWS * W)),
            hal=np.ones((128, 2), np.float32) * np.array(
                [0.0 if h0 == 0 else 1.0, 0.0 if h0 + ROWS == H else 1.0],
                np.float32)[None, :],
            segg=np.ascontiguousarray(segp),
            maskg=np.ascontiguousarray(maskp),
            codes=np.ascontiguousarray(codes_vector[b]),
        ))

    res = run_bass_kernel_spmd(_NC, in_maps, list(range(NCORES)))

    out = np.empty((B, C, H, W), np.float32)
    for c in range(NCORES):
        b, half = divmod(c, 2)
        h0 = half * ROWS
        out[b, :, h0:h0 + ROWS, :] = res.results[c]["out"].reshape(C, ROWS, W)
    return out



# revision 13
# speedup vs baseline: 1.5523x; 1.5523x over previous
"""Trainium2 Bass kernel for nn_Decoder_22196390985918 (SPADE-style decoder).

Sharding: 8 cores = (batch b in 0..3) x (H-half in 0..1). Each core computes
out[b, :, h0:h0+64, :] for h0 = 64*(core%2).

The [B, 512, H, W] "middle" tensor (masked scatter of per-region style
vectors mu[b,j,:]) is never materialized: conv(middle) collapses to a conv
over the 5 one-hot region masks sel_j with per-batch tap tables
G[j, cc, tap] = sum_k Wconv[cc, k, tap] * mu[b, j, k], i.e. one K=45 matmul
per output tile on top of the 9 K=128 SPADE tap matmuls.

All heavy matmul operands are bf16. Weight transposes and the sigmoid
blending factors are folded on the host: the device receives ready-to-use
lhsT layouts (wct [k,tap,cc], spT [nh,tap,cc], sswT [27,nh], fcwT [l,k]) and
a single combined per-channel bias. mu is computed on the PE as 80 small
K=128/N=1 matmuls (which also warm up the PE p-state ramp), then the region
count masks (cnt), the shared conv and the main conv run as one continuous
PE stream with the cnt/shared chunks interleaved between main-conv chunks.
The beta half of the conv output is combined via a cross-partition DVE add
(no PE shift matmul). Instance-norm stats come from the core's own x half
(f32) plus the other half loaded as bf16.
"""
import os as _os

import numpy as np
import ml_dtypes

import concourse.bacc as bacc
import concourse.bass as bass
import concourse.mybir as mybir
import concourse.tile as tile
from concourse.bass_utils import run_bass_kernel_spmd

dt = mybir.dt
F32 = dt.float32
BF16 = dt.bfloat16
AF = mybir.ActivationFunctionType
ALU = mybir.AluOpType
NPBF = ml_dtypes.bfloat16

B, C, H, W, F, L, NH = 4, 64, 128, 128, 5, 512, 128
GW = 130                    # padded grid width  (image col = grid col - 1)
SR = 66                     # seg/sel/actv grid rows (image row = h0 - 1 + r)
MR = 68                     # mask grid rows (image row = h0 - 2 + r)
SEG_N = SR * GW             # 8580
MASK_N = MR * GW            # 8840
ROWS = 64                   # output rows per core
NCH = 16                    # main conv chunks (4 rows x 128 cols, N=512)
ACH = 22                    # shared conv chunks (3 rows x 128 cols, N=384)
NCORES = 8
CNTC = 17                   # cnt/sel chunks of 512 cols over SEG_N


def _build_nc():
    lvl = int(_os.environ.get("KSEC", "9"))
    nc = bacc.Bacc()

    # ---- per-core DRAM inputs -------------------------------------------
    xown_d = nc.dram_tensor("xown", [C, ROWS * W], F32, kind="ExternalInput")
    xoth_d = nc.dram_tensor("xoth", [C, ROWS * W], BF16, kind="ExternalInput")
    segg = nc.dram_tensor("segg", [F, SEG_N + 264], BF16, kind="ExternalInput")
    maskg = nc.dram_tensor("maskg", [3, MASK_N + 264], BF16, kind="ExternalInput")
    codesT_d = nc.dram_tensor("codesT", [L, F], BF16, kind="ExternalInput")
    fcbT_d = nc.dram_tensor("fcbT", [L, F], F32, kind="ExternalInput")
    fcwT_d = nc.dram_tensor("fcwT", [F, L * L], BF16, kind="ExternalInput")
    wct_d = nc.dram_tensor("wct", [L, 9 * 128], BF16, kind="ExternalInput")
    spT_d = nc.dram_tensor("spT", [NH, 9 * 128], BF16, kind="ExternalInput")
    sswT_d = nc.dram_tensor("sswT", [27, NH], BF16, kind="ExternalInput")
    u5_d = nc.dram_tensor("u5", [45, 45], BF16, kind="ExternalInput")
    bias1_d = nc.dram_tensor("bias1", [128, 1], F32, kind="ExternalInput")
    ssb_d = nc.dram_tensor("ssb", [NH, 1], F32, kind="ExternalInput")
    hal_d = nc.dram_tensor("hal", [128, 2], F32, kind="ExternalInput")
    out_d = nc.dram_tensor("out", [C, NCH, 512], F32, kind="ExternalOutput")

    segp = segg[:].ap[0][0]     # dram row stride (elements)
    maskp = maskg[:].ap[0][0]

    with tile.TileContext(nc) as tc:
        with (
            tc.tile_pool(name="const", bufs=1) as cst,
            tc.tile_pool(name="gb", bufs=3) as gbp,
            tc.tile_pool(name="ot", bufs=3) as otp,
            tc.tile_pool(name="pmain", bufs=3, space="PSUM") as pmain,
            tc.tile_pool(name="paux", bufs=2, space="PSUM") as paux,
            tc.tile_pool(name="gpsp", bufs=3, space="PSUM") as gpsp,
        ):
            # ---- tiny constants (sync queue head) -----------------------
            u5r = cst.tile([45, 45], BF16)
            nc.sync.dma_start(out=u5r[:], in_=u5_d[:])
            sswT = cst.tile([27, NH], BF16)
            nc.sync.dma_start(out=sswT[:], in_=sswT_d[:])
            codesT = cst.tile([128, 4, F], BF16)
            nc.sync.dma_start(
                out=codesT[:],
                in_=bass.AP(tensor=codesT_d[:].tensor, offset=0,
                            ap=[[F, 128], [128 * F, 4], [1, F]]))
            fcbT = cst.tile([128, 4, F], F32)
            nc.sync.dma_start(
                out=fcbT[:],
                in_=bass.AP(tensor=fcbT_d[:].tensor, offset=0,
                            ap=[[F, 128], [128 * F, 4], [1, F]]))
            bias1_t = cst.tile([128, 1], F32)
            nc.sync.dma_start(out=bias1_t[:], in_=bias1_d[:])
            ssb_t = cst.tile([NH, 1], F32)
            nc.sync.dma_start(out=ssb_t[:], in_=ssb_d[:])
            hal_t = cst.tile([128, 2], F32)
            nc.sync.dma_start(out=hal_t[:], in_=hal_d[:])

            eps_t = cst.tile([C, 1], F32)
            nc.gpsimd.memset(eps_t[:], 1e-5)
            zsb = cst.tile([128, 132], BF16)
            nc.gpsimd.memset(zsb[:], 0.0)
            ones45 = cst.tile([45, 1], F32)
            nc.gpsimd.memset(ones45[:], 1.0)

            # ---- fcwT loads: j0,j2,j4 on scalar q; j1,j3 on sync q ------
            # layout per j: ft[l(128 part), lb(4), k(512)]; DMA split by k
            # half so the (j, kb) matmul order can start early.
            fts = []
            for j in range(F):
                ft = cst.tile([128, 4, L], BF16, name=f"ft{j}")
                fts.append(ft)
            for j in [0, 1, 2, 3, 4]:
                eng = nc.scalar if j % 2 == 0 else nc.sync
                for kh in range(2):
                    src = bass.AP(tensor=fcwT_d[:].tensor,
                                  offset=j * L * L + kh * 256,
                                  ap=[[L, 128], [128 * L, 4], [1, 256]])
                    eng.dma_start(out=fts[j][:, :, kh * 256:(kh + 1) * 256],
                                  in_=src)

            # ---- xoth (stats) on scalar q after fcwT --------------------
            xoth = cst.tile([C, ROWS * W], BF16)
            for h in range(2):
                nc.scalar.dma_start(
                    out=xoth[:, h * 4096:(h + 1) * 4096],
                    in_=xoth_d[:, h * 4096:(h + 1) * 4096])

            # ---- wct + spT on sync q ------------------------------------
            wcts = []
            for kb in range(4):
                wct = cst.tile([128, 9, 128], BF16, name=f"wct{kb}")
                nc.sync.dma_start(
                    out=wct[:],
                    in_=wct_d[kb * 128:(kb + 1) * 128, :].rearrange(
                        "k (t c) -> k t c", t=9))
                wcts.append(wct)
            spT = cst.tile([128, 9, 128], BF16)
            nc.sync.dma_start(out=spT[:],
                              in_=spT_d[:].rearrange("k (t c) -> k t c", t=9))

            # ---- grids: pre-shifted replicated loads (gpsimd queue) -----
            sel45 = cst.tile([45, SEG_N], BF16)
            for ty in range(3):
                for ch in range(2):
                    c0 = ch * 4290
                    src = bass.AP(tensor=segg[:].tensor, offset=ty * GW + c0,
                                  ap=[[1, 3], [segp, F], [1, 4290]])
                    nc.gpsimd.dma_start(
                        out=sel45[15 * ty:15 * ty + 15, c0:c0 + 4290], in_=src)
            mask27 = cst.tile([27, MASK_N], BF16)
            for ty in range(3):
                src = bass.AP(tensor=maskg[:].tensor, offset=ty * GW,
                              ap=[[1, 3], [maskp, 3], [1, MASK_N]])
                nc.gpsimd.dma_start(out=mask27[9 * ty:9 * ty + 9, :], in_=src)

            # ---- xown on gpsimd q after grids ---------------------------
            xown = cst.tile([C, ROWS * W], F32)
            for h in range(4):
                nc.gpsimd.dma_start(
                    out=xown[:, h * 2048:(h + 1) * 2048],
                    in_=xown_d[:, h * 2048:(h + 1) * 2048])

            # ---- mu on PE: 80 small matmuls, warms the p-state ramp -----
            if lvl >= 2:
                pz = gpsp.tile([128, 4, F], F32, tag="gps", name="pz")
                for j in range(F):
                    for kb in range(4):
                        for lb in range(4):
                            nc.tensor.matmul(
                                pz[:, kb, j:j + 1],
                                fts[j][:, lb, kb * 128:(kb + 1) * 128],
                                codesT[:, lb, j:j + 1],
                                start=(lb == 0), stop=(lb == 3))
                z2 = cst.tile([128, 4, F], F32)
                nc.vector.tensor_add(z2[:], pz[:], fcbT[:])
                muT = cst.tile([128, 4, F], BF16)
                nc.scalar.activation(muT[:], z2[:], AF.Relu)

            # ---- G matmuls -> selG (shift DMAs on scalar q) -------------
            if lvl >= 3:
                gps = [gpsp.tile([F, 3, 128], F32, tag="gps", name=f"gps{g}")
                       for g in range(3)]
                for kb in range(4):
                    for g in range(3):
                        nc.tensor.matmul(gps[g][:], muT[:, kb, :],
                                         wcts[kb][:, 3 * g:3 * g + 3, :],
                                         start=(kb == 0), stop=(kb == 3))
                gstage = cst.tile([F, 9, 128], BF16)
                for g in range(3):
                    nc.scalar.activation(gstage[:, 3 * g:3 * g + 3, :],
                                         gps[g][:], AF.Copy)
                selG = cst.tile([45, 128], BF16)
                for t in range(9):
                    nc.scalar.dma_start(out=selG[F * t:F * t + F, :],
                                        in_=gstage[:, t, :])

            # ---- instance-norm stats (vector) ---------------------------
            if lvl >= 4:
                stats_t = cst.tile([C, 32, 6], F32)
                for q in range(16):
                    nc.vector.bn_stats(out=stats_t[:, q, :],
                                       in_=xoth[:, q * 512:(q + 1) * 512])
                for q in range(16):
                    nc.vector.bn_stats(out=stats_t[:, 16 + q, :],
                                       in_=xown[:, q * 512:(q + 1) * 512])
                mv = cst.tile([C, 2], F32)
                nc.vector.bn_aggr(out=mv[:], in_=stats_t[:])
                sd = cst.tile([C, 1], F32)
                nc.scalar.activation(sd[:], mv[:, 1:2], AF.Sqrt,
                                     bias=eps_t[:], scale=1.0)
                rstd = cst.tile([C, 1], F32)
                nc.vector.reciprocal(rstd[:], sd[:])
                nbias = cst.tile([C, 1], F32)
                nc.vector.tensor_mul(nbias[:], mv[:, 0:1], rstd[:])
                nc.vector.tensor_scalar_mul(nbias[:], nbias[:], -1.0)

            # ---- aux chunk emitters (interleaved with main conv) --------
            t_sb = cst.tile([45, SEG_N], BF16)
            actv = cst.tile([NH, SR, GW], BF16)
            if lvl >= 5:
                # zero border cols 0 and 129 of actv
                bord = actv[:, :, 0:1]
                nc.vector.tensor_copy(
                    bass.AP(tensor=bord.tensor, offset=bord.offset,
                            ap=[bord.ap[0], [GW, SR], [GW - 1, 2]]),
                    zsb[:].rearrange("p (a b) -> p a b", a=SR))
            m3 = mask27[:].rearrange("p (r c) -> p r c", c=GW)
            s3 = sel45[:].rearrange("p (r c) -> p r c", c=GW)

            segchunks = []
            off = 0
            while off < SEG_N:
                n = min(512, SEG_N - off)
                segchunks.append((off, n))
                off += n

            def cnt_chunk(c):
                off, n = segchunks[c]
                pc = paux.tile([45, 512], F32, tag="aux", name=f"cnt{c}")
                nc.tensor.matmul(pc[:, 0:n], u5r[:], sel45[:, off:off + n],
                                 start=True, stop=True)
                # t = relu(1 - cnt); then sel *= t (both exact in bf16)
                nc.scalar.activation(t_sb[:, off:off + n], pc[:, 0:n],
                                     AF.Relu, bias=ones45[:], scale=-1.0)

            def mult_chunk(c):
                off, n = segchunks[c]
                nc.vector.tensor_mul(sel45[:, off:off + n],
                                     sel45[:, off:off + n],
                                     t_sb[:, off:off + n])

            def shared_chunk(a):
                r = 3 * a
                psh = paux.tile([NH, 3, 128], F32, tag="aux", name=f"sh{a}")
                nc.tensor.matmul(psh[:], sswT[:], m3[:, r:r + 3, 0:128],
                                 start=True, stop=True)
                nc.scalar.activation(actv[:, r:r + 3, 1:129], psh[:],
                                     AF.Relu, bias=ssb_t[:], scale=1.0)

            def hal_fix_row(a):
                # zero out-of-image halo rows of actv (reference zero-pads)
                if a == 0:
                    nc.vector.tensor_scalar_mul(actv[:, 0, :], actv[:, 0, :],
                                                hal_t[:, 0:1])
                else:
                    nc.vector.tensor_scalar_mul(actv[:, SR - 1, :],
                                                actv[:, SR - 1, :],
                                                hal_t[:, 1:2])

            # ---- main conv + epilogue -----------------------------------
            if lvl >= 6:
                pms = {}

                def conv_chunk(i):
                    pm = pmain.tile([128, 4, 128], F32, tag="pm",
                                    name=f"pm{i}")
                    pms[i] = pm
                    for t in range(9):
                        ty, tx = divmod(t, 3)
                        nc.tensor.matmul(
                            pm[:], spT[:, t, :],
                            actv[:, 4 * i + ty:4 * i + ty + 4, tx:tx + 128],
                            start=(t == 0), stop=False)
                    nc.tensor.matmul(pm[:], selG[:],
                                     s3[:, 4 * i:4 * i + 4, 0:128],
                                     start=False, stop=True)

                def epi_chunk(i):
                    pm = pms.pop(i)
                    gb = gbp.tile([128, 512], F32, tag="gb", name=f"gb{i}")
                    nc.scalar.activation(
                        gb[:], pm[:].rearrange("p t c -> p (t c)"),
                        AF.Identity, bias=bias1_t[:], scale=1.0)
                    xnt = otp.tile([C, 512], F32, tag="ot", name=f"xnt{i}")
                    nc.gpsimd.tensor_scalar(xnt[:],
                                            xown[:, i * 512:(i + 1) * 512],
                                            rstd[:], nbias[:],
                                            op0=ALU.mult, op1=ALU.add)
                    nc.gpsimd.tensor_mul(xnt[:], xnt[:], gb[0:64, :])
                    # beta lives on partitions 64:128 -> shift down via DMA
                    # (compute engines require equal operand base partitions)
                    pb = otp.tile([C, 512], F32, tag="ot", name=f"pb{i}")
                    nc.scalar.dma_start(out=pb[:], in_=gb[64:128, :])
                    nc.vector.tensor_add(xnt[:], xnt[:], pb[:])
                    nc.sync.dma_start(out=out_d[:, i, :], in_=xnt[:])

                cnt_done = 0
                sh_done = 0
                for i in range(NCH):
                    need_cnt = min(CNTC, (520 * i + 518) // 512 + 1)
                    need_sh = min(ACH, (4 * i + 6) // 3 + 1)
                    while cnt_done < need_cnt:
                        cnt_chunk(cnt_done)
                        mult_chunk(cnt_done)
                        cnt_done += 1
                    while sh_done < need_sh:
                        shared_chunk(sh_done)
                        if sh_done == 0 or sh_done == ACH - 1:
                            hal_fix_row(sh_done)
                        sh_done += 1
                    conv_chunk(i)
                    if i > 0:
                        epi_chunk(i - 1)
                # drain remaining aux chunks (cols/rows beyond last image row)
                while cnt_done < CNTC:
                    cnt_chunk(cnt_done)
                    mult_chunk(cnt_done)
                    cnt_done += 1
                while sh_done < ACH:
                    shared_chunk(sh_done)
                    if sh_done == ACH - 1:
                        hal_fix_row(sh_done)
                    sh_done += 1
                epi_chunk(NCH - 1)

    nc.finalize()
    return nc


_NC = None


def _make_in_maps(inputs):
    x = np.asarray(inputs["x"], dtype=np.float32)
    segmap = np.asarray(inputs["segmap"], dtype=np.float32)
    codes_vector = np.asarray(inputs["codes_vector"], dtype=np.float32)
    mask = np.asarray(inputs["mask"], dtype=np.float32)
    fc_w = np.asarray(inputs["fc_w"], dtype=np.float32)
    fc_b = np.asarray(inputs["fc_b"], dtype=np.float32)
    conv_gamma_w = np.asarray(inputs["conv_gamma_w"], dtype=np.float32)
    conv_gamma_b = np.asarray(inputs["conv_gamma_b"], dtype=np.float32)
    conv_beta_w = np.asarray(inputs["conv_beta_w"], dtype=np.float32)
    conv_beta_b = np.asarray(inputs["conv_beta_b"], dtype=np.float32)
    spade_shared_w = np.asarray(inputs["spade_shared_w"], dtype=np.float32)
    spade_shared_b = np.asarray(inputs["spade_shared_b"], dtype=np.float32)
    spade_gamma_w = np.asarray(inputs["spade_gamma_w"], dtype=np.float32)
    spade_gamma_b = np.asarray(inputs["spade_gamma_b"], dtype=np.float32)
    spade_beta_w = np.asarray(inputs["spade_beta_w"], dtype=np.float32)
    spade_beta_b = np.asarray(inputs["spade_beta_b"], dtype=np.float32)
    blending_gamma = np.asarray(inputs["blending_gamma"], dtype=np.float32)
    blending_beta = np.asarray(inputs["blending_beta"], dtype=np.float32)

    ga = 1.0 / (1.0 + np.exp(-float(blending_gamma[0])))
    ba = 1.0 / (1.0 + np.exp(-float(blending_beta[0])))

    # combined conv weights, blend folded in, transposed to lhsT layouts
    wc = np.concatenate([ga * conv_gamma_w, ba * conv_beta_w], axis=0)
    wct = wc.transpose(1, 2, 3, 0).reshape(L, 9 * 128)        # [k,(t,cc)]
    sp = np.concatenate([(1.0 - ga) * spade_gamma_w,
                         (1.0 - ba) * spade_beta_w], axis=0)
    spT = sp.transpose(1, 2, 3, 0).reshape(NH, 9 * 128)       # [nh,(t,cc)]
    sswT = spade_shared_w.transpose(0, 2, 3, 1).reshape(NH, 27).T  # [27,nh]
    bias1 = np.concatenate([
        ga * conv_gamma_b + (1.0 - ga) * spade_gamma_b + 1.0,
        ba * conv_beta_b + (1.0 - ba) * spade_beta_b]).reshape(128, 1)
    fcwT = fc_w.transpose(0, 2, 1).reshape(F, L * L)          # [j, (l, k)]
    u5 = np.kron(np.eye(9, dtype=np.float32),
                 np.tril(np.ones((F, F), np.float32), -1))

    shared = {
        "fcwT": np.ascontiguousarray(fcwT).astype(NPBF),
        "fcbT": np.ascontiguousarray(fc_b.T),
        "wct": np.ascontiguousarray(wct).astype(NPBF),
        "spT": np.ascontiguousarray(spT).astype(NPBF),
        "sswT": np.ascontiguousarray(sswT).astype(NPBF),
        "u5": u5.astype(NPBF),
        "bias1": bias1.astype(np.float32),
        "ssb": spade_shared_b.reshape(NH, 1).astype(np.float32),
    }

    in_maps = []
    for c in range(NCORES):
        b, half = divmod(c, 2)
        h0 = half * ROWS
        segp = np.zeros((F, SEG_N + 264), NPBF)
        segp2 = np.zeros((F, SR, GW), np.float32)
        r_lo, r_hi = h0 - 1, h0 + ROWS + 1  # exclusive
        s_lo, s_hi = max(r_lo, 0), min(r_hi, H)
        segp2[:, s_lo - r_lo:s_hi - r_lo, 1:129] = segmap[b, :, s_lo:s_hi, :]
        segp[:, 0:SEG_N] = segp2.reshape(F, -1).astype(NPBF)
        maskp = np.zeros((3, MASK_N + 264), NPBF)
        maskp2 = np.zeros((3, MR, GW), np.float32)
        m_lo, m_hi = h0 - 2, h0 + ROWS + 2
        ms_lo, ms_hi = max(m_lo, 0), min(m_hi, H)
        maskp2[:, ms_lo - m_lo:ms_hi - m_lo, 1:129] = mask[b, :, ms_lo:ms_hi, :]
        maskp[:, 0:MASK_N] = maskp2.reshape(3, -1).astype(NPBF)
        oh0 = ROWS - h0  # other half start
        in_maps.append(dict(
            shared,
            xown=np.ascontiguousarray(
                x[b, :, h0:h0 + ROWS, :].reshape(C, ROWS * W)),
            xoth=np.ascontiguousarray(
                x[b, :, oh0:oh0 + ROWS, :].reshape(C, ROWS * W)).astype(NPBF),
            hal=np.ones((128, 2), np.float32) * np.array(
                [0.0 if h0 == 0 else 1.0, 0.0 if h0 + ROWS == H else 1.0],
                np.float32)[None, :],
            segg=np.ascontiguousarray(segp),
            maskg=np.ascontiguousarray(maskp),
            codesT=np.ascontiguousarray(codes_vector[b].T).astype(NPBF),
        ))
    return in_maps


def kernel(**inputs):
    global _NC
    if _NC is None:
        _NC = _build_nc()
    in_maps = _make_in_maps(inputs)
    res = run_bass_kernel_spmd(_NC, in_maps, list(range(NCORES)))

    out = np.empty((B, C, H, W), np.float32)
    for c in range(NCORES):
        b, half = divmod(c, 2)
        h0 = half * ROWS
        out[b, :, h0:h0 + ROWS, :] = res.results[c]["out"].reshape(C, ROWS, W)
    return out


# revision 15
# speedup vs baseline: 1.8199x; 1.1724x over previous
"""Trainium2 Bass kernel for nn_Decoder_22196390985918 (SPADE-style decoder).

Sharding: 8 cores = (batch b in 0..3) x (H-half in 0..1). Each core computes
out[b, :, h0:h0+64, :] for h0 = 64*(core%2).

The [B, 512, H, W] "middle" tensor (masked scatter of per-region style
vectors mu[b,j,:]) is never materialized: conv(middle) collapses to a conv
over the 5 one-hot region masks sel_j with per-batch tap tables
G[j, cc, tap] = sum_k Wconv[cc, k, tap] * mu[b, j, k], i.e. one K=45 matmul
per output tile on top of the 9 K=128 SPADE tap matmuls.

All heavy matmul operands are bf16. Weight transposes and the sigmoid
blending factors are folded on the host; every large DMA is a plain 2D
transfer with per-partition-contiguous source spans (3-dim APs cost multi-us
descriptor generation on the issuing engine). Tiny constants ride in two
packed tensors. mu is computed on the PE as 80 small K=128/N=1 matmuls that
warm up the PE p-state ramp, then cnt masks, the shared conv and the main
conv run as one PE stream with aux chunks interleaved between main chunks.
gamma/beta leave PSUM via two half-height ACTs (the beta ACT reads PSUM
partitions 64:128 and writes partitions 0:64, so no shift matmul/DMA).
Instance-norm stats come from a bf16 copy of the full image; the epilogue
multiplies the core's own f32 half.
"""
import os as _os

import numpy as np
import ml_dtypes

import concourse.bacc as bacc
import concourse.bass as bass
import concourse.mybir as mybir
import concourse.tile as tile
from concourse.bass_utils import run_bass_kernel_spmd

dt = mybir.dt
F32 = dt.float32
BF16 = dt.bfloat16
AF = mybir.ActivationFunctionType
ALU = mybir.AluOpType
NPBF = ml_dtypes.bfloat16

B, C, H, W, F, L, NH = 4, 64, 128, 128, 5, 512, 128
GW = 130                    # padded grid width  (image col = grid col - 1)
SR = 66                     # seg/sel/actv grid rows (image row = h0 - 1 + r)
MR = 68                     # mask grid rows (image row = h0 - 2 + r)
SEG_N = SR * GW             # 8580
MASK_N = MR * GW            # 8840
ROWS = 64                   # output rows per core
NCH = 16                    # main conv chunks (4 rows x 128 cols, N=512)
ACH = 22                    # shared conv chunks (3 rows x 128 cols, N=384)
NCORES = 8
CNTC = 17                   # cnt/sel chunks of 512 cols over SEG_N
PKF = 25                    # f32 const pack cols
PKB = 193                   # bf16 const pack cols


def _build_nc():
    lvl = int(_os.environ.get("KSEC", "9"))
    nc = bacc.Bacc()

    # ---- per-core DRAM inputs -------------------------------------------
    xown_d = nc.dram_tensor("xown", [C, ROWS * W], F32, kind="ExternalInput")
    xful_d = nc.dram_tensor("xful", [C, H * W], BF16, kind="ExternalInput")
    segg = nc.dram_tensor("segg", [F, SEG_N + 264], BF16, kind="ExternalInput")
    maskg = nc.dram_tensor("maskg", [3, MASK_N + 264], BF16,
                           kind="ExternalInput")
    fcwT_d = nc.dram_tensor("fcwT", [128, F * 4 * L], BF16,
                            kind="ExternalInput")
    wct_d = nc.dram_tensor("wct", [L, 9 * 128], BF16, kind="ExternalInput")
    spT_d = nc.dram_tensor("spT", [NH, 9 * 128], BF16, kind="ExternalInput")
    pkf_d = nc.dram_tensor("pkf", [128, PKF], F32, kind="ExternalInput")
    pkb_d = nc.dram_tensor("pkb", [128, PKB], BF16, kind="ExternalInput")
    out_d = nc.dram_tensor("out", [C, NCH, 512], F32, kind="ExternalOutput")

    segp = segg[:].ap[0][0]     # dram row stride (elements)
    maskp = maskg[:].ap[0][0]

    with tile.TileContext(nc) as tc:
        with (
            tc.tile_pool(name="const", bufs=1) as cst,
            tc.tile_pool(name="gg", bufs=3) as ggp,
            tc.tile_pool(name="bb", bufs=3) as bbp,
            tc.tile_pool(name="ot", bufs=3) as otp,
            tc.tile_pool(name="pmain", bufs=3, space="PSUM") as pmain,
            tc.tile_pool(name="paux", bufs=2, space="PSUM") as paux,
            tc.tile_pool(name="gpsp", bufs=3, space="PSUM") as gpsp,
        ):
            # ---- const packs (2 DMAs on sync) ---------------------------
            pkf = cst.tile([128, PKF], F32)
            nc.sync.dma_start(out=pkf[:], in_=pkf_d[:])
            pkb = cst.tile([128, PKB], BF16)
            nc.sync.dma_start(out=pkb[:], in_=pkb_d[:])
            fcbT = pkf[:, 0:20].rearrange("p (l j) -> p l j", l=4)
            bias1g = pkf[0:64, 20:21]
            bias1b = pkf[0:64, 21:22]
            ssb_t = pkf[:, 22:23]
            hal_t = pkf[:, 23:25]
            u5r = pkb[0:45, 0:45]
            sswT = pkb[0:27, 45:173]
            codesT = pkb[:, 173:193].rearrange("p (l j) -> p l j", l=4)

            eps_t = cst.tile([C, 1], F32)
            nc.gpsimd.memset(eps_t[:], 1e-5)
            zsb = cst.tile([128, 132], BF16)
            nc.gpsimd.memset(zsb[:], 0.0)
            ones45 = cst.tile([45, 1], F32)
            nc.gpsimd.memset(ones45[:], 1.0)

            # ---- fcwT: plain 2D loads, j0/j2/j4 scalar, j1/j3 sync ------
            ftall = cst.tile([128, F, 4, L], BF16)
            for j in range(F):
                eng = nc.scalar if j % 2 == 0 else nc.sync
                eng.dma_start(out=ftall[:, j, :, :].rearrange("p l k -> p (l k)"),
                              in_=fcwT_d[:, j * 4 * L:(j + 1) * 4 * L])

            # ---- wct + spT on sync q (plain 2D) -------------------------
            wcts = []
            for kb in range(4):
                wct = cst.tile([128, 9 * 128], BF16, name=f"wct{kb}")
                nc.sync.dma_start(out=wct[:],
                                  in_=wct_d[kb * 128:(kb + 1) * 128, :])
                wcts.append(wct[:].rearrange("p (t c) -> p t c", t=9))
            spT_f = cst.tile([128, 9 * 128], BF16)
            nc.sync.dma_start(out=spT_f[:], in_=spT_d[:])
            spT = spT_f[:].rearrange("p (t c) -> p t c", t=9)

            # ---- grids: pre-shifted replicated loads (gpsimd queue) -----
            sel45 = cst.tile([45, SEG_N], BF16)
            for ty in range(3):
                src = bass.AP(tensor=segg[:].tensor, offset=ty * GW,
                              ap=[[1, 3], [segp, F], [1, SEG_N]])
                nc.gpsimd.dma_start(out=sel45[15 * ty:15 * ty + 15, :],
                                    in_=src)
            mask27 = cst.tile([27, MASK_N], BF16)
            for ty in range(3):
                src = bass.AP(tensor=maskg[:].tensor, offset=ty * GW,
                              ap=[[1, 3], [maskp, 3], [1, MASK_N]])
                nc.gpsimd.dma_start(out=mask27[9 * ty:9 * ty + 9, :], in_=src)

            # ---- x loads: bf16 full image (scalar), f32 own half (sync) -
            xful = cst.tile([C, H * W], BF16)
            for h in range(2):
                nc.scalar.dma_start(out=xful[:, h * 8192:(h + 1) * 8192],
                                    in_=xful_d[:, h * 8192:(h + 1) * 8192])
            xown = cst.tile([C, ROWS * W], F32)
            for h in range(2):
                nc.sync.dma_start(out=xown[:, h * 4096:(h + 1) * 4096],
                                  in_=xown_d[:, h * 4096:(h + 1) * 4096])

            # ---- mu on PE: 80 small matmuls, warms the p-state ramp -----
            if lvl >= 2:
                pz = gpsp.tile([128, 4, F], F32, tag="gps", name="pz")
                for j in range(F):
                    for kb in range(4):
                        for lb in range(4):
                            nc.tensor.matmul(
                                pz[:, kb, j:j + 1],
                                ftall[:, j, lb, kb * 128:(kb + 1) * 128],
                                codesT[:, lb, j:j + 1],
                                start=(lb == 0), stop=(lb == 3))
                z2 = cst.tile([128, 4, F], F32)
                nc.vector.tensor_add(z2[:], pz[:], fcbT)
                muT = cst.tile([128, 4, F], BF16)
                nc.scalar.activation(muT[:], z2[:], AF.Relu)

            # ---- G matmuls -> selG (shift DMAs on scalar q) -------------
            if lvl >= 3:
                gps = [gpsp.tile([F, 3, 128], F32, tag="gps", name=f"gps{g}")
                       for g in range(3)]
                for kb in range(4):
                    for g in range(3):
                        nc.tensor.matmul(gps[g][:], muT[:, kb, :],
                                         wcts[kb][:, 3 * g:3 * g + 3, :],
                                         start=(kb == 0), stop=(kb == 3))
                gstage = cst.tile([F, 9, 128], BF16)
                for g in range(3):
                    nc.scalar.activation(gstage[:, 3 * g:3 * g + 3, :],
                                         gps[g][:], AF.Copy)
                selG = cst.tile([45, 128], BF16)
                for t in range(9):
                    nc.scalar.dma_start(out=selG[F * t:F * t + F, :],
                                        in_=gstage[:, t, :])

            # ---- instance-norm stats from bf16 full image (vector) ------
            if lvl >= 4:
                stats_t = cst.tile([C, 32, 6], F32)
                for q in range(32):
                    nc.vector.bn_stats(out=stats_t[:, q, :],
                                       in_=xful[:, q * 512:(q + 1) * 512])
                mv = cst.tile([C, 2], F32)
                nc.vector.bn_aggr(out=mv[:], in_=stats_t[:])
                sd = cst.tile([C, 1], F32)
                nc.scalar.activation(sd[:], mv[:, 1:2], AF.Sqrt,
                                     bias=eps_t[:], scale=1.0)
                rstd = cst.tile([C, 1], F32)
                nc.vector.reciprocal(rstd[:], sd[:])
                nbias = cst.tile([C, 1], F32)
                nc.vector.tensor_mul(nbias[:], mv[:, 0:1], rstd[:])
                nc.vector.tensor_scalar_mul(nbias[:], nbias[:], -1.0)

            # ---- aux chunk emitters (interleaved with main conv) --------
            t_sb = cst.tile([45, SEG_N], BF16)
            actv = cst.tile([NH, SR, GW], BF16)
            if lvl >= 5:
                # zero border cols 0 and 129 of actv
                bord = actv[:, :, 0:1]
                nc.vector.tensor_copy(
                    bass.AP(tensor=bord.tensor, offset=bord.offset,
                            ap=[bord.ap[0], [GW, SR], [GW - 1, 2]]),
                    zsb[:].rearrange("p (a b) -> p a b", a=SR))
            m3 = mask27[:].rearrange("p (r c) -> p r c", c=GW)
            s3 = sel45[:].rearrange("p (r c) -> p r c", c=GW)

            segchunks = []
            off = 0
            while off < SEG_N:
                n = min(512, SEG_N - off)
                segchunks.append((off, n))
                off += n

            def cnt_chunk(c):
                off, n = segchunks[c]
                pc = paux.tile([45, 512], F32, tag="aux", name=f"cnt{c}")
                nc.tensor.matmul(pc[:, 0:n], u5r, sel45[:, off:off + n],
                                 start=True, stop=True)
                # t = relu(1 - cnt); then sel *= t (both exact in bf16)
                nc.scalar.activation(t_sb[:, off:off + n], pc[:, 0:n],
                                     AF.Relu, bias=ones45[:], scale=-1.0)

            def mult_chunk(c):
                off, n = segchunks[c]
                nc.vector.tensor_mul(sel45[:, off:off + n],
                                     sel45[:, off:off + n],
                                     t_sb[:, off:off + n])

            def shared_chunk(a):
                r = 3 * a
                psh = paux.tile([NH, 3, 128], F32, tag="aux", name=f"sh{a}")
                nc.tensor.matmul(psh[:], sswT, m3[:, r:r + 3, 0:128],
                                 start=True, stop=True)
                nc.scalar.activation(actv[:, r:r + 3, 1:129], psh[:],
                                     AF.Relu, bias=ssb_t, scale=1.0)

            def hal_fix_row(a):
                # zero out-of-image halo rows of actv (reference zero-pads)
                if a == 0:
                    nc.vector.tensor_scalar_mul(actv[:, 0, :], actv[:, 0, :],
                                                hal_t[:, 0:1])
                else:
                    nc.vector.tensor_scalar_mul(actv[:, SR - 1, :],
                                                actv[:, SR - 1, :],
                                                hal_t[:, 1:2])

            # ---- main conv + epilogue -----------------------------------
            if lvl >= 6:
                pms = {}

                def conv_chunk(i):
                    pm = pmain.tile([128, 4, 128], F32, tag="pm",
                                    name=f"pm{i}")
                    pms[i] = pm
                    for t in range(9):
                        ty, tx = divmod(t, 3)
                        nc.tensor.matmul(
                            pm[:], spT[:, t, :],
                            actv[:, 4 * i + ty:4 * i + ty + 4, tx:tx + 128],
                            start=(t == 0), stop=False)
                    nc.tensor.matmul(pm[:], selG[:],
                                     s3[:, 4 * i:4 * i + 4, 0:128],
                                     start=False, stop=True)

                def epi_chunk(i):
                    pm = pms.pop(i)
                    pmf = pm[:].rearrange("p t c -> p (t c)")
                    gg = ggp.tile([C, 512], F32, tag="gg", name=f"gg{i}")
                    nc.scalar.activation(gg[:], pmf[0:64, :], AF.Identity,
                                         bias=bias1g, scale=1.0)
                    bb = bbp.tile([C, 512], F32, tag="bb", name=f"bb{i}")
                    nc.scalar.activation(bb[:], pmf[64:128, :], AF.Identity,
                                         bias=bias1b, scale=1.0)
                    xnt = otp.tile([C, 512], F32, tag="ot", name=f"xnt{i}")
                    nc.gpsimd.tensor_scalar(xnt[:],
                                            xown[:, i * 512:(i + 1) * 512],
                                            rstd[:], nbias[:],
                                            op0=ALU.mult, op1=ALU.add)
                    nc.gpsimd.tensor_mul(xnt[:], xnt[:], gg[:])
                    nc.vector.tensor_add(xnt[:], xnt[:], bb[:])
                    nc.sync.dma_start(out=out_d[:, i, :], in_=xnt[:])

                cnt_done = 0
                sh_done = 0
                for i in range(NCH):
                    need_cnt = min(CNTC, (520 * i + 518) // 512 + 1)
                    need_sh = min(ACH, (4 * i + 6) // 3 + 1)
                    while cnt_done < need_cnt:
                        cnt_chunk(cnt_done)
                        mult_chunk(cnt_done)
                        cnt_done += 1
                    while sh_done < need_sh:
                        shared_chunk(sh_done)
                        if sh_done == 0 or sh_done == ACH - 1:
                            hal_fix_row(sh_done)
                        sh_done += 1
                    conv_chunk(i)
                    if i > 0:
                        epi_chunk(i - 1)
                while cnt_done < CNTC:
                    cnt_chunk(cnt_done)
                    mult_chunk(cnt_done)
                    cnt_done += 1
                while sh_done < ACH:
                    shared_chunk(sh_done)
                    if sh_done == ACH - 1:
                        hal_fix_row(sh_done)
                    sh_done += 1
                epi_chunk(NCH - 1)

    nc.finalize()
    return nc


_NC = None


def _make_in_maps(inputs):
    x = np.asarray(inputs["x"], dtype=np.float32)
    segmap = np.asarray(inputs["segmap"], dtype=np.float32)
    codes_vector = np.asarray(inputs["codes_vector"], dtype=np.float32)
    mask = np.asarray(inputs["mask"], dtype=np.float32)
    fc_w = np.asarray(inputs["fc_w"], dtype=np.float32)
    fc_b = np.asarray(inputs["fc_b"], dtype=np.float32)
    conv_gamma_w = np.asarray(inputs["conv_gamma_w"], dtype=np.float32)
    conv_gamma_b = np.asarray(inputs["conv_gamma_b"], dtype=np.float32)
    conv_beta_w = np.asarray(inputs["conv_beta_w"], dtype=np.float32)
    conv_beta_b = np.asarray(inputs["conv_beta_b"], dtype=np.float32)
    spade_shared_w = np.asarray(inputs["spade_shared_w"], dtype=np.float32)
    spade_shared_b = np.asarray(inputs["spade_shared_b"], dtype=np.float32)
    spade_gamma_w = np.asarray(inputs["spade_gamma_w"], dtype=np.float32)
    spade_gamma_b = np.asarray(inputs["spade_gamma_b"], dtype=np.float32)
    spade_beta_w = np.asarray(inputs["spade_beta_w"], dtype=np.float32)
    spade_beta_b = np.asarray(inputs["spade_beta_b"], dtype=np.float32)
    blending_gamma = np.asarray(inputs["blending_gamma"], dtype=np.float32)
    blending_beta = np.asarray(inputs["blending_beta"], dtype=np.float32)

    ga = 1.0 / (1.0 + np.exp(-float(blending_gamma[0])))
    ba = 1.0 / (1.0 + np.exp(-float(blending_beta[0])))

    # combined conv weights, blend folded in, transposed to lhsT layouts
    wc = np.concatenate([ga * conv_gamma_w, ba * conv_beta_w], axis=0)
    wct = wc.transpose(1, 2, 3, 0).reshape(L, 9 * 128)        # [k,(t,cc)]
    sp = np.concatenate([(1.0 - ga) * spade_gamma_w,
                         (1.0 - ba) * spade_beta_w], axis=0)
    spT = sp.transpose(1, 2, 3, 0).reshape(NH, 9 * 128)       # [nh,(t,cc)]
    sswT = spade_shared_w.transpose(0, 2, 3, 1).reshape(NH, 27).T  # [27,nh]
    # fcwT host layout: [p(128), j, lb, k] so each partition's data is one
    # contiguous DRAM span (descriptor-cheap 2D DMA)
    fcwT = np.ascontiguousarray(
        fc_w.transpose(0, 2, 1).reshape(F, 4, 128, L)
        .transpose(2, 0, 1, 3).reshape(128, F * 4 * L))

    # f32 const pack: fcbT(20) | bias1g | bias1b | ssb | hal(2)
    pkf = np.zeros((128, PKF), np.float32)
    pkf[:, 0:20] = fc_b.T.reshape(4, 128, F).transpose(1, 0, 2).reshape(128, 20)
    pkf[0:64, 20] = ga * conv_gamma_b + (1.0 - ga) * spade_gamma_b + 1.0
    pkf[0:64, 21] = ba * conv_beta_b + (1.0 - ba) * spade_beta_b
    pkf[:, 22] = spade_shared_b
    u5 = np.kron(np.eye(9, dtype=np.float32),
                 np.tril(np.ones((F, F), np.float32), -1))

    shared = {
        "fcwT": fcwT.astype(NPBF),
        "wct": np.ascontiguousarray(wct).astype(NPBF),
        "spT": np.ascontiguousarray(spT).astype(NPBF),
    }

    in_maps = []
    for c in range(NCORES):
        b, half = divmod(c, 2)
        h0 = half * ROWS
        segp = np.zeros((F, SEG_N + 264), NPBF)
        segp2 = np.zeros((F, SR, GW), np.float32)
        r_lo, r_hi = h0 - 1, h0 + ROWS + 1  # exclusive
        s_lo, s_hi = max(r_lo, 0), min(r_hi, H)
        segp2[:, s_lo - r_lo:s_hi - r_lo, 1:129] = segmap[b, :, s_lo:s_hi, :]
        segp[:, 0:SEG_N] = segp2.reshape(F, -1).astype(NPBF)
        maskp = np.zeros((3, MASK_N + 264), NPBF)
        maskp2 = np.zeros((3, MR, GW), np.float32)
        m_lo, m_hi = h0 - 2, h0 + ROWS + 2
        ms_lo, ms_hi = max(m_lo, 0), min(m_hi, H)
        maskp2[:, ms_lo - m_lo:ms_hi - m_lo, 1:129] = mask[b, :, ms_lo:ms_hi, :]
        maskp[:, 0:MASK_N] = maskp2.reshape(3, -1).astype(NPBF)
        pkfc = pkf.copy()
        pkfc[:, 23] = 0.0 if h0 == 0 else 1.0
        pkfc[:, 24] = 0.0 if h0 + ROWS == H else 1.0
        # bf16 const pack: u5(45) | sswT(128) | codesT(20)
        pkb = np.zeros((128, PKB), NPBF)
        pkb[0:45, 0:45] = u5.astype(NPBF)
        pkb[0:27, 45:173] = sswT.astype(NPBF)
        pkb[:, 173:193] = (codes_vector[b].T.reshape(4, 128, F)
                           .transpose(1, 0, 2).reshape(128, 20).astype(NPBF))
        in_maps.append(dict(
            shared,
            xown=np.ascontiguousarray(
                x[b, :, h0:h0 + ROWS, :].reshape(C, ROWS * W)),
            xful=np.ascontiguousarray(x[b].reshape(C, H * W)).astype(NPBF),
            pkf=pkfc,
            pkb=pkb,
            segg=np.ascontiguousarray(segp),
            maskg=np.ascontiguousarray(maskp),
        ))
    return in_maps


def kernel(**inputs):
    global _NC
    if _NC is None:
        _NC = _build_nc()
    in_maps = _make_in_maps(inputs)
    res = run_bass_kernel_spmd(_NC, in_maps, list(range(NCORES)))

    out = np.empty((B, C, H, W), np.float32)
    for c in range(NCORES):
        b, half = divmod(c, 2)
        h0 = half * ROWS
        out[b, :, h0:h0 + ROWS, :] = res.results[c]["out"].reshape(C, ROWS, W)
    return out
